# revision 18
# baseline (speedup 1.0000x reference)
"""Trainium2 Bass kernel for nn_MultiHeadAttention_9912784519532.

MHA with relative position bias: b=2, n=2048, dim=512, heads=8, d_head=64,
rel table (2*512+1, 64).

Sharding: 16 (batch, head) pairs over 8 cores -> 2 heads of one batch per
core. Each core computes a partial output y_part = attn_out @ Wo_slice for
its 2 heads; host sums 4 partials per batch and adds bo.

Per-core algorithm (keys-on-partitions / transposed-attention orientation):
  qT/kT = W.T @ x.T via PE (2 heads packed on partitions 0-63 / 64-127)
  kT_past/kT_fut = kT + rel_emb[1024/0]  (folds the clipped far-field
    positional bias into the S^T matmul exactly)
  wER[n, c] = q[n] . rel_emb[clip(1152 - c, 0, 1024)]  (reversed+edge-padded
    relative projection table) -> DRAM scratch, one tensor per (head, q-tile)
  For each (head, q-chunk 1024, key-tile 128):
    Z^T = kT_variant.T @ qT  (class per 128-col block: past/window/future)
    window blocks: Z^T += transpose-matmul of diagonally-DMA'd wER tiles
      (pos[n, r] = wER[n, 640 - n + r] is a plain 2D-strided DRAM read)
    attnT = exp(0.125 * Z^T)  (ScalarE; logits are O(1), no max needed)
    outT_aug += [v | 1].T @ attnT  (row 64 accumulates the softmax denom)
  outT = outT_aug[:64] / outT_aug[64]; y_part = outT.T @ Wo_slice

Scheduling notes (the point of this revision):
- wER DRAM scratch is split per (head, q-tile): a flash window DMA reads
  exactly one q-tile's rows, so it only waits on that tile's write.
- wER chunk matmuls are PSUM->SBUF-cast paced (the slow engines); emitting
  them contiguously blocks the in-order PE queue on pool-slot recycling.
  Instead they are interleaved into the flash kt loop as background tasks,
  so each cast has a whole flash iteration to drain.
- The softmax division's PSUM-releasing copies run on whichever of
  Vector/Scalar is not congested at that point in the schedule.
"""

import numpy as np

HEADS = 8
D = 64
N = 2048
DIM = 512
WER = 1280  # width of padded/reversed rel projection table
P = 128

_cached = {}


def _build_program():
    import concourse.bass as bass
    import concourse.mybir as mybir
    import concourse.tile as tile
    from concourse import bacc

    f32 = mybir.dt.float32
    bf16 = mybir.dt.bfloat16
    AP = bass.AP

    nc = bacc.Bacc(
        "TRN2",
        target_bir_lowering=False,
        debug=False,
        enable_asserts=False,
        num_devices=8,
    )

    xT_d = nc.dram_tensor("xT", [DIM, N], bf16, kind="ExternalInput")
    wq_d = nc.dram_tensor("wq2", [DIM, P], bf16, kind="ExternalInput")
    wk_d = nc.dram_tensor("wk2", [DIM, P], bf16, kind="ExternalInput")
    wv_d = nc.dram_tensor("wv2", [DIM, P], bf16, kind="ExternalInput")
    wo_d = nc.dram_tensor("wo2", [P, DIM], bf16, kind="ExternalInput")
    relx_d = nc.dram_tensor("relx2", [P, WER], bf16, kind="ExternalInput")
    edge_d = nc.dram_tensor("edge2", [P, 2], f32, kind="ExternalInput")
    ident_d = nc.dram_tensor("ident", [P, P], f32, kind="ExternalInput")
    ones_d = nc.dram_tensor("ones1", [1, 64], bf16, kind="ExternalInput")
    y_d = nc.dram_tensor("y", [N, DIM], f32, kind="ExternalOutput")

    SCALE = float(D) ** -0.5
    NT = N // P  # 16 key tiles
    QW = 1024  # q-chunk width for the flash loop

    # per-(head, q-tile) scratch: window DMAs depend only on their own tile
    wer_d = [
        [nc.dram_tensor(f"wer{h}_{qt}", [P, WER], bf16, kind="Internal")
         for qt in range(NT)]
        for h in range(2)
    ]

    with tile.TileContext(nc) as tc:
        import contextlib

        ctx = contextlib.ExitStack()
        with ctx:
            const = ctx.enter_context(tc.tile_pool(name="const", bufs=1))
            big = ctx.enter_context(tc.tile_pool(name="big", bufs=1))
            cpool = ctx.enter_context(tc.tile_pool(name="copies", bufs=4))
            # PSUM: pp 2 + zs 3 + os 3 = 8 banks
            pp = ctx.enter_context(tc.tile_pool(name="pp", bufs=2, space="PSUM"))
            zs = ctx.enter_context(tc.tile_pool(name="zs", bufs=3, space="PSUM"))
            os_ = ctx.enter_context(tc.tile_pool(name="os", bufs=3, space="PSUM"))
            apool = ctx.enter_context(tc.tile_pool(name="attn", bufs=3))
            wpool = ctx.enter_context(tc.tile_pool(name="win", bufs=16))
            spool = ctx.enter_context(tc.tile_pool(name="small", bufs=4))

            # ---- constants with no input deps ----
            v2 = big.tile([P, 2, NT, 65], bf16)
            nc.vector.memset(v2[:], 1.0)
            ones1 = const.tile([1, 64], bf16)
            nc.sync.dma_start(ones1[:], ones_d.ap())

            # ---- load inputs (xT split per chunk so projections start early)
            xt_sb = []
            for cc in range(4):
                xt = big.tile([P, 4, 512], bf16, name=f"xt{cc}", tag=f"xt{cc}")
                for nch in range(4):
                    nc.sync.dma_start(
                        xt[:, nch, :],
                        xT_d.ap()[cc * P : (cc + 1) * P,
                                  nch * 512 : (nch + 1) * 512],
                    )
                xt_sb.append(xt)
            wq_sb = const.tile([P, 4, P], bf16)
            nc.sync.dma_start(wq_sb[:], wq_d.ap().rearrange("(c p) m -> p c m", p=P))
            wk_sb = const.tile([P, 4, P], bf16)
            nc.sync.dma_start(wk_sb[:], wk_d.ap().rearrange("(c p) m -> p c m", p=P))
            wv_sb = const.tile([P, 4, P], bf16)
            nc.sync.dma_start(wv_sb[:], wv_d.ap().rearrange("(c p) m -> p c m", p=P))
            wo_sb = const.tile([64, 2, DIM], bf16)
            nc.sync.dma_start(wo_sb[:], wo_d.ap().rearrange("(h p) m -> p h m", p=64))
            relx_sb = const.tile([P, WER], bf16)
            nc.sync.dma_start(relx_sb[:], relx_d.ap())
            edge_sb = const.tile([P, 2], f32)
            nc.sync.dma_start(edge_sb[:], edge_d.ap())
            ident_sb = const.tile([P, P], f32)
            nc.sync.dma_start(ident_sb[:], ident_d.ap())

            # ---- projections: qT2/kT2 (2 heads packed on partitions) ----
            qt2 = big.tile([P, N], bf16)
            kt2 = big.tile([P, N], bf16)
            for di, (dst, wsb) in enumerate(((qt2, wq_sb), (kt2, wk_sb))):
                for nch in range(4):
                    pt = pp.tile([P, 512], f32, name="proj", tag="ps")
                    for cc in range(4):
                        nc.tensor.matmul(
                            pt[:],
                            wsb[:, cc, :],
                            xt_sb[cc][:, nch, :],
                            start=(cc == 0),
                            stop=(cc == 3),
                        )
                    if (nch + di) % 2 == 0:
                        nc.vector.tensor_copy(
                            dst[:, nch * 512 : (nch + 1) * 512], pt[:]
                        )
                    else:
                        nc.scalar.copy(dst[:, nch * 512 : (nch + 1) * 512], pt[:])

            ktp = big.tile([P, N], bf16)
            ktf = big.tile([P, N], bf16)
            nc.scalar.add(ktp[:], kt2[:], edge_sb[:, 0:1])
            nc.vector.tensor_scalar_add(ktf[:], kt2[:], edge_sb[:, 1:2])

            # ---- v (natural, keys on partitions), packed as [v | 1] ----
            for kt in range(NT):
                pt = pp.tile([P, 512], f32, name="vproj", tag="ps")
                for cc in range(4):
                    nc.tensor.matmul(
                        pt[:, :P],
                        xt_sb[cc][:, kt // 4, (kt % 4) * P : (kt % 4) * P + P],
                        wv_sb[:, cc, :],
                        start=(cc == 0),
                        stop=(cc == 3),
                    )
                nc.vector.tensor_copy(v2[:, 0, kt, 0:64], pt[:, 0:64])
                nc.scalar.copy(v2[:, 1, kt, 0:64], pt[:, 64:128])

            # ---- wER chunk task: 3 matmuls + 2 casts + DMA out ----
            def wer_task(h, qt, during_flash):
                hs = slice(h * 64, h * 64 + 64)
                wtile = cpool.tile([P, WER], bf16, name="wer_sb", tag="wer_sb")
                for ci, (c0, cw) in enumerate(((0, 512), (512, 512), (1024, 256))):
                    pt = pp.tile([P, 512], f32, name="wer_ps", tag="ps")
                    nc.tensor.matmul(
                        pt[:, :cw],
                        qt2[hs, qt * P : (qt + 1) * P],
                        relx_sb[hs, c0 : c0 + cw],
                        start=True,
                        stop=True,
                    )
                    # during flash ScalarE is exp-bound: keep casts on Vector
                    use_v = ci != 2 if during_flash else (ci + qt) % 2 == 0
                    if use_v:
                        nc.vector.tensor_copy(wtile[:, c0 : c0 + cw], pt[:, :cw])
                    else:
                        nc.scalar.copy(wtile[:, c0 : c0 + cw], pt[:, :cw])
                nc.sync.dma_start(wer_d[h][qt].ap(), wtile[:])

            # ---- flash attention ----
            otn = big.tile([64, 2, N], bf16)  # normalized outT per head

            def emit_division(h, qc, oth, eng):
                # oth: two [65, 512] PSUM tiles (outT_aug halves). Row 64 is
                # the softmax denominator; normalize rows 0..63 into otn.
                # eng: engine for the PSUM-releasing copies (pick the one
                # with the shallower queue at this point in the schedule).
                for half in range(2):
                    q0 = qc * QW + half * 512
                    dsb = spool.tile([1, 512], f32, name="dsb", tag="dsb")
                    stage = spool.tile([64, 512], f32, name="ostg", tag="ostg")
                    if eng == "v":
                        nc.vector.tensor_copy(dsb[:], oth[half][64:65, :])
                        nc.vector.tensor_copy(stage[:], oth[half][0:64, :])
                    else:
                        nc.scalar.copy(dsb[:], oth[half][64:65, :])
                        nc.scalar.copy(stage[:], oth[half][0:64, :])
                    # recip_approx from PSUM mis-executes on HW; SBUF is exact
                    rdenf = spool.tile([1, 512], f32, name="rdenf", tag="rdenf")
                    nc.vector.reciprocal_approx_fast(rdenf[:], dsb[:])
                    rdenb = spool.tile([1, 512], bf16, name="rdenb", tag="rdenb")
                    nc.gpsimd.tensor_copy(rdenb[:], rdenf[:])
                    rcb = pp.tile([64, 512], f32, name="rcb", tag="ps")
                    nc.tensor.matmul(
                        rcb[:], ones1[:], rdenb[:], start=True, stop=True
                    )
                    nc.vector.tensor_mul(
                        otn[:, h, q0 : q0 + 512], stage[:], rcb[:]
                    )

            def emit_outproj(nt_range):
                for nt in nt_range:
                    pt = pp.tile([P, 512], f32, name="yproj", tag="ps")
                    for h in range(2):
                        nc.tensor.matmul(
                            pt[:],
                            otn[:, h, nt * P : (nt + 1) * P],
                            wo_sb[:, h, :],
                            start=(h == 0),
                            stop=(h == 1),
                        )
                    yt = cpool.tile([P, 512], f32, name="y_sb", tag="y_sb")
                    if nt % 2 == 0:
                        nc.vector.tensor_copy(yt[:], pt[:])
                    else:
                        nc.scalar.copy(yt[:], pt[:])
                    nc.sync.dma_start(y_d.ap()[nt * P : (nt + 1) * P, :], yt[:])

            def emit_flash(h, qc, bg_tasks):
                """bg_tasks: list of (wer_h, qt) chunk tasks to interleave.
                For qc==0 windows j reads wer[h][j]; tasks for this h's qt<=4
                must already be done; the rest of this h's tasks must be at
                the FRONT of bg_tasks in qt order (windows are emitted as
                their tiles become available)."""
                wins = [None] * (QW // P)

                def emit_win(j):
                    qb = qc * QW + j * P
                    r0 = max(0, qb - 512)
                    r1 = min(N, qb + 640)
                    rw = r1 - r0
                    wt = wpool.tile([P, 1152], f32, name=f"win{j}", tag="win")
                    wsrc = AP(
                        tensor=wer_d[h][qc * (QW // P) + j],
                        offset=640 + r0 - qb,
                        ap=[[WER - 1, P], [1, rw]],
                    )
                    nc.gpsimd.dma_start(wt[:, :rw], wsrc)
                    wins[j] = (wt, r0)

                own_pending = [t for t in bg_tasks if t[0] == h]
                done_qt = set(range(NT)) - {t[1] for t in own_pending}

                def win_ready(j):
                    return (qc * (QW // P) + j) in done_qt

                def drain_until(j):
                    while not win_ready(j):
                        assert bg_tasks, f"window {j} has no pending wER task"
                        th, tq = bg_tasks.pop(0)
                        wer_task(th, tq, during_flash=True)
                        if th == h:
                            done_qt.add(tq)

                # emit windows available now; others as their tiles complete
                for j in range(QW // P):
                    if win_ready(j):
                        emit_win(j)

                oth = [
                    os_.tile([65, 512], f32, name=f"outT{half}", tag="outT")
                    for half in range(2)
                ]
                prev_at = None
                for kt in range(NT):
                    kb = kt * P
                    cls = []
                    for j in range(QW // P):
                        dlt = qc * QW + j * P - kb
                        cls.append(
                            "p" if dlt >= 640 else ("f" if dlt <= -640 else "w")
                        )
                    # windows needed this kt: emit any missing (draining bg)
                    for j in range(QW // P):
                        if cls[j] == "w" and wins[j] is None:
                            drain_until(j)
                            emit_win(j)
                    zh = []
                    for half in range(QW // 512):
                        j0 = half * 4
                        zt = zs.tile([P, 512], f32, name="zt", tag="zt")
                        runs = []
                        for j in range(j0, j0 + 4):
                            if runs and runs[-1][2] == cls[j]:
                                runs[-1][1] += P
                            else:
                                runs.append([(j - j0) * P, P, cls[j]])
                        first = True
                        for s, wd, c in runs:
                            kvar = {"p": ktp, "f": ktf, "w": kt2}[c]
                            nc.tensor.matmul(
                                zt[:, s : s + wd],
                                kvar[hs_of(h), kb : kb + P],
                                qt2[hs_of(h), qc * QW + j0 * P + s :
                                    qc * QW + j0 * P + s + wd],
                                start=first,
                                stop=False,
                                skip_group_check=True,
                            )
                            first = False
                        for j in range(j0, j0 + 4):
                            if cls[j] != "w":
                                continue
                            wt, r0 = wins[j]
                            nc.tensor.matmul(
                                zt[:, (j - j0) * P : (j - j0 + 1) * P],
                                wt[:, kb - r0 : kb - r0 + P],
                                ident_sb[:],
                                is_transpose=True,
                                start=False,
                                stop=False,
                                skip_group_check=True,
                            )
                        zh.append(zt)
                    at = apool.tile([P, QW], bf16, name="attnT", tag="attnT")
                    for half in range(2):
                        nc.scalar.activation(
                            at[:, half * 512 : (half + 1) * 512],
                            zh[half][:],
                            mybir.ActivationFunctionType.Exp,
                            scale=SCALE,
                        )
                    # lag AV by one kt so PE never stalls on the current exp
                    if prev_at is not None:
                        pat, pkt = prev_at
                        for half in range(2):
                            nc.tensor.matmul(
                                oth[half][:],
                                v2[:, h, pkt, :],
                                pat[:, half * 512 : (half + 1) * 512],
                                start=(pkt == 0),
                                stop=False,
                            )
                    prev_at = (at, kt)
                    # interleave background wER work between flash steps
                    if bg_tasks:
                        th, tq = bg_tasks.pop(0)
                        wer_task(th, tq, during_flash=True)
                        if th == h:
                            done_qt.add(tq)
                pat, pkt = prev_at
                for half in range(2):
                    nc.tensor.matmul(
                        oth[half][:],
                        v2[:, h, pkt, :],
                        pat[:, half * 512 : (half + 1) * 512],
                        start=False,
                        stop=True,
                    )
                return oth

            def hs_of(h):
                return slice(h * 64, h * 64 + 64)

            # wER h0 qt0..4 up front (flash(0,0) kt0 needs windows j0..4)
            for qt in range(5):
                wer_task(0, qt, during_flash=False)
            bg = [(0, qt) for qt in range(5, NT)]
            oth = emit_flash(0, 0, bg)
            emit_division(0, 0, oth, "v")
            bg = [(1, qt) for qt in range(NT)]
            oth = emit_flash(0, 1, bg)
            emit_division(0, 1, oth, "s")
            oth = emit_flash(1, 0, bg)
            emit_division(1, 0, oth, "v")
            emit_outproj(range(0, 8))  # rows 0..1023 ready; overlaps last flash
            oth = emit_flash(1, 1, bg)
            emit_division(1, 1, oth, "s")
            emit_outproj(range(8, 16))

    nc.compile()
    return nc


def _host_prep(x, Wq, Wkv, Wo, rel_emb):
    """Build the 8 per-core input maps."""
    import ml_dtypes

    bf = ml_dtypes.bfloat16
    ident = np.eye(P, dtype=np.float32)
    ones1 = np.ones((1, 64), dtype=bf)
    relX = rel_emb[np.clip(1152 - np.arange(WER), 0, 1024)].T
    relx2 = np.ascontiguousarray(np.concatenate([relX, relX], axis=0).astype(bf))
    edge = np.stack([rel_emb[1024], rel_emb[0]], axis=1)
    edge2 = np.ascontiguousarray(np.concatenate([edge, edge], axis=0).astype(np.float32))
    Wkv_r = Wkv.reshape(DIM, 2, HEADS, D)
    in_maps = []
    for core in range(8):
        b = core // 4
        h0 = 2 * (core % 4)
        in_maps.append(
            {
                "xT": np.ascontiguousarray(x[b].T.astype(bf)),
                "wq2": np.ascontiguousarray(Wq[:, h0 * D : (h0 + 2) * D].astype(bf)),
                "wk2": np.ascontiguousarray(
                    Wkv_r[:, 0, h0 : h0 + 2].reshape(DIM, 2 * D).astype(bf)
                ),
                "wv2": np.ascontiguousarray(
                    Wkv_r[:, 1, h0 : h0 + 2].reshape(DIM, 2 * D).astype(bf)
                ),
                "wo2": np.ascontiguousarray(
                    Wo[h0 * D : (h0 + 2) * D, :].astype(bf)
                ),
                "relx2": relx2,
                "edge2": edge2,
                "ident": ident,
                "ones1": ones1,
            }
        )
    return in_maps


def kernel(x, Wq, Wkv, Wo, bo, rel_emb, _want_trace=False):
    from concourse.bass_utils import run_bass_kernel_spmd

    x = np.asarray(x)
    if "nc" not in _cached:
        _cached["nc"] = _build_program()
    nc = _cached["nc"]
    in_maps = _host_prep(x, np.asarray(Wq), np.asarray(Wkv), np.asarray(Wo),
                         np.asarray(rel_emb))
    res = run_bass_kernel_spmd(
        nc, in_maps, core_ids=list(range(8)), trace=_want_trace
    )
    _cached["last_result"] = res
    y = np.zeros((2, N, DIM), np.float32)
    for core in range(8):
        y[core // 4] += res.results[core]["y"]
    y += np.asarray(bo).astype(np.float32)[None, None, :]
    return y


# revision 26
# speedup vs baseline: 1.0512x; 1.0512x over previous
"""Trainium2 Bass kernel for nn_MultiHeadAttention_9912784519532.

MHA with relative position bias: b=2, n=2048, dim=512, heads=8, d_head=64,
rel table (2*512+1, 64).

Sharding: 16 (batch, head) pairs over 8 cores -> 2 heads of one batch per
core. Each core computes a partial output y_part = attn_out @ Wo_slice for
its 2 heads; host sums 4 partials per batch and adds bo.

Per-core algorithm (keys-on-partitions / transposed-attention orientation):
  qT/kT = W.T @ x.T via PE (2 heads packed on partitions 0-63 / 64-127)
  kT_past/kT_fut = kT + rel_emb[1024/0]  (folds the clipped far-field
    positional bias into the S^T matmul exactly)
  wER[n, c] = q[n] . rel_emb[clip(1152 - c, 0, 1024)]  (reversed+edge-padded
    relative projection table) -> DRAM scratch, one tensor per (head, q-tile)
  For each (head, q-chunk 1024, key-tile 128):
    Z^T = kT_variant.T @ qT  (class per 128-col block: past/window/future)
    window blocks: Z^T += transpose-matmul of diagonally-DMA'd wER tiles
      (pos[n, r] = wER[n, 640 - n + r] is a plain 2D-strided DRAM read)
    attnT = exp(0.125 * Z^T)  (ScalarE; logits are O(1), no max needed)
    outT_aug += [v | 1].T @ attnT  (row 64 accumulates the softmax denom)
  outT = outT_aug[:64] / outT_aug[64]; y_part = outT.T @ Wo_slice

Scheduling notes (the point of this revision):
- wER DRAM scratch is split per (head, q-tile): a flash window DMA reads
  exactly one q-tile's rows, so it only waits on that tile's write.
- wER chunk matmuls are PSUM->SBUF-cast paced (the slow engines); emitting
  them contiguously blocks the in-order PE queue on pool-slot recycling.
  Instead they are interleaved into the flash kt loop as background tasks,
  so each cast has a whole flash iteration to drain.
- The softmax division's PSUM-releasing copies run on whichever of
  Vector/Scalar is not congested at that point in the schedule.
"""

import numpy as np

HEADS = 8
D = 64
N = 2048
DIM = 512
WER = 1280  # width of padded/reversed rel projection table
P = 128

_cached = {}


def _build_program():
    import concourse.bass as bass
    import concourse.mybir as mybir
    import concourse.tile as tile
    from concourse import bacc

    f32 = mybir.dt.float32
    bf16 = mybir.dt.bfloat16
    AP = bass.AP

    nc = bacc.Bacc(
        "TRN2",
        target_bir_lowering=False,
        debug=False,
        enable_asserts=False,
        num_devices=8,
    )

    xT_d = nc.dram_tensor("xT", [DIM, N], bf16, kind="ExternalInput")
    wq_d = nc.dram_tensor("wq2", [DIM, P], bf16, kind="ExternalInput")
    wk_d = nc.dram_tensor("wk2", [DIM, P], bf16, kind="ExternalInput")
    wv_d = nc.dram_tensor("wv2", [DIM, P], bf16, kind="ExternalInput")
    wo_d = nc.dram_tensor("wo2", [P, DIM], bf16, kind="ExternalInput")
    relx_d = nc.dram_tensor("relx2", [P, WER], bf16, kind="ExternalInput")
    edge_d = nc.dram_tensor("edge2", [P, 2], f32, kind="ExternalInput")
    ident_d = nc.dram_tensor("ident", [P, P], bf16, kind="ExternalInput")
    ones_d = nc.dram_tensor("ones1", [1, 64], bf16, kind="ExternalInput")
    y_d = nc.dram_tensor("y", [N, DIM], f32, kind="ExternalOutput")

    SCALE = float(D) ** -0.5
    NT = N // P  # 16 key tiles
    QW = 1024  # q-chunk width for the flash loop

    # per-(head, q-tile) scratch: window DMAs depend only on their own tile
    wer_d = [
        [nc.dram_tensor(f"wer{h}_{qt}", [P, WER], bf16, kind="Internal")
         for qt in range(NT)]
        for h in range(2)
    ]

    with tile.TileContext(nc) as tc:
        import contextlib

        ctx = contextlib.ExitStack()
        with ctx:
            const = ctx.enter_context(tc.tile_pool(name="const", bufs=1))
            big = ctx.enter_context(tc.tile_pool(name="big", bufs=1))
            cpool = ctx.enter_context(tc.tile_pool(name="copies", bufs=4))
            # PSUM: pp 2 + zs 3 + os 3 = 8 banks
            pp = ctx.enter_context(tc.tile_pool(name="pp", bufs=2, space="PSUM"))
            zs = ctx.enter_context(tc.tile_pool(name="zs", bufs=3, space="PSUM"))
            os_ = ctx.enter_context(tc.tile_pool(name="os", bufs=3, space="PSUM"))
            apool = ctx.enter_context(tc.tile_pool(name="attn", bufs=3))
            wpool = ctx.enter_context(tc.tile_pool(name="win", bufs=16))
            spool = ctx.enter_context(tc.tile_pool(name="small", bufs=4))

            # ---- constants with no input deps ----
            v2 = big.tile([P, 2, NT, 65], bf16)
            nc.vector.memset(v2[:], 1.0)
            ones1 = const.tile([1, 64], bf16)
            nc.sync.dma_start(ones1[:], ones_d.ap())

            # ---- load inputs; spread descriptor issue over the three
            # DMA-capable queues (sync, scalar, gpsimd) so startup isn't
            # serialized on the sync engine
            xt_sb = []
            for cc in range(4):
                xt = big.tile([P, 4, 512], bf16, name=f"xt{cc}", tag=f"xt{cc}")
                eng = (nc.sync, nc.scalar, nc.gpsimd, nc.sync)[cc]
                eng.dma_start(
                    xt[:].rearrange("p a b -> p (a b)"),
                    xT_d.ap()[cc * P : (cc + 1) * P, :],
                )
                xt_sb.append(xt)
            wq_sb = const.tile([P, 4, P], bf16)
            nc.scalar.dma_start(wq_sb[:], wq_d.ap().rearrange("(c p) m -> p c m", p=P))
            wk_sb = const.tile([P, 4, P], bf16)
            nc.gpsimd.dma_start(wk_sb[:], wk_d.ap().rearrange("(c p) m -> p c m", p=P))
            wv_sb = const.tile([P, 4, P], bf16)
            nc.sync.dma_start(wv_sb[:], wv_d.ap().rearrange("(c p) m -> p c m", p=P))
            wo_sb = const.tile([P, DIM], bf16)
            nc.scalar.dma_start(wo_sb[:], wo_d.ap())
            relx_sb = const.tile([P, WER], bf16)
            nc.gpsimd.dma_start(relx_sb[:], relx_d.ap())
            edge_sb = const.tile([P, 2], f32)
            nc.sync.dma_start(edge_sb[:], edge_d.ap())
            ident_sb = const.tile([P, P], bf16)
            nc.scalar.dma_start(ident_sb[:], ident_d.ap())

            # ---- projections: qT2/kT2 (2 heads packed on partitions) ----
            qt2 = big.tile([P, N], bf16)
            kt2 = big.tile([P, N], bf16)
            for di, (dst, wsb) in enumerate(((qt2, wq_sb), (kt2, wk_sb))):
                for nch in range(4):
                    pt = pp.tile([P, 512], f32, name="proj", tag="ps")
                    for cc in range(4):
                        nc.tensor.matmul(
                            pt[:],
                            wsb[:, cc, :],
                            xt_sb[cc][:, nch, :],
                            start=(cc == 0),
                            stop=(cc == 3),
                        )
                    if (nch + di) % 2 == 0:
                        nc.vector.tensor_copy(
                            dst[:, nch * 512 : (nch + 1) * 512], pt[:]
                        )
                    else:
                        nc.scalar.copy(dst[:, nch * 512 : (nch + 1) * 512], pt[:])

            ktp = big.tile([P, N], bf16)
            ktf = big.tile([P, N], bf16)
            nc.scalar.add(ktp[:], kt2[:], edge_sb[:, 0:1])
            nc.vector.tensor_scalar_add(ktf[:], kt2[:], edge_sb[:, 1:2])

            # ---- v (natural, keys on partitions), packed as [v | 1] ----
            for kt in range(NT):
                pt = pp.tile([P, 512], f32, name="vproj", tag="ps")
                for cc in range(4):
                    nc.tensor.matmul(
                        pt[:, :P],
                        xt_sb[cc][:, kt // 4, (kt % 4) * P : (kt % 4) * P + P],
                        wv_sb[:, cc, :],
                        start=(cc == 0),
                        stop=(cc == 3),
                    )
                nc.vector.tensor_copy(v2[:, 0, kt, 0:64], pt[:, 0:64])
                nc.scalar.copy(v2[:, 1, kt, 0:64], pt[:, 64:128])

            # ---- wER chunk task: 3 matmuls + 2 casts + DMA out ----
            def wer_task(h, qt, during_flash):
                hs = slice(h * 64, h * 64 + 64)
                wtile = cpool.tile([P, WER], bf16, name="wer_sb", tag="wer_sb")
                for ci, (c0, cw) in enumerate(((0, 512), (512, 512), (1024, 256))):
                    pt = pp.tile([P, 512], f32, name="wer_ps", tag="ps")
                    nc.tensor.matmul(
                        pt[:, :cw],
                        qt2[hs, qt * P : (qt + 1) * P],
                        relx_sb[hs, c0 : c0 + cw],
                        start=True,
                        stop=True,
                    )
                    # during flash ScalarE is exp-bound: keep casts on Vector
                    use_v = ci != 2 if during_flash else (ci + qt) % 2 == 0
                    if use_v:
                        nc.vector.tensor_copy(wtile[:, c0 : c0 + cw], pt[:, :cw])
                    else:
                        nc.scalar.copy(wtile[:, c0 : c0 + cw], pt[:, :cw])
                nc.sync.dma_start(wer_d[h][qt].ap(), wtile[:])

            # ---- flash attention ----
            # normalized outT, h0 on partitions 0..63, h1 on 64..127 — the
            # output projection is then a single K=128 matmul per n-tile
            otn = big.tile([P, N], bf16)

            def emit_division(h, qc, oth, eng):
                # oth: two [65, 512] PSUM tiles (outT_aug halves). Row 64 is
                # the softmax denominator; normalize rows 0..63 into otn.
                # eng: engine for the PSUM-releasing copies (pick the one
                # with the shallower queue at this point in the schedule).
                for half in range(2):
                    q0 = qc * QW + half * 512
                    dsb = spool.tile([1, 512], f32, name="dsb", tag="dsb")
                    stage = spool.tile([64, 512], f32, name="ostg", tag="ostg")
                    if eng == "v":
                        nc.vector.tensor_copy(dsb[:], oth[half][64:65, :])
                        nc.vector.tensor_copy(stage[:], oth[half][0:64, :])
                    else:
                        nc.scalar.copy(dsb[:], oth[half][64:65, :])
                        nc.scalar.copy(stage[:], oth[half][0:64, :])
                    # recip_approx from PSUM mis-executes on HW; SBUF is exact
                    rdenf = spool.tile([1, 512], f32, name="rdenf", tag="rdenf")
                    nc.vector.reciprocal_approx_fast(rdenf[:], dsb[:])
                    rdenb = spool.tile([1, 512], bf16, name="rdenb", tag="rdenb")
                    nc.gpsimd.tensor_copy(rdenb[:], rdenf[:])
                    rcb = pp.tile([64, 512], f32, name="rcb", tag="ps")
                    nc.tensor.matmul(
                        rcb[:], ones1[:], rdenb[:], start=True, stop=True
                    )
                    nc.vector.tensor_mul(
                        otn[h * 64 : h * 64 + 64, q0 : q0 + 512],
                        stage[:], rcb[:]
                    )

            def emit_outproj(nt_range):
                for nt in nt_range:
                    pt = pp.tile([P, 512], f32, name="yproj", tag="ps")
                    nc.tensor.matmul(
                        pt[:],
                        otn[:, nt * P : (nt + 1) * P],
                        wo_sb[:],
                        start=True,
                        stop=True,
                    )
                    yt = cpool.tile([P, 512], f32, name="y_sb", tag="y_sb")
                    if nt % 2 == 0:
                        nc.vector.tensor_copy(yt[:], pt[:])
                        nc.sync.dma_start(
                            y_d.ap()[nt * P : (nt + 1) * P, :], yt[:]
                        )
                    else:
                        nc.scalar.copy(yt[:], pt[:])
                        nc.scalar.dma_start(
                            y_d.ap()[nt * P : (nt + 1) * P, :], yt[:]
                        )

            def emit_flash(h, qc, bg_tasks):
                """bg_tasks: list of (wer_h, qt) chunk tasks to interleave.
                For qc==0 windows j reads wer[h][j]; tasks for this h's qt<=4
                must already be done; the rest of this h's tasks must be at
                the FRONT of bg_tasks in qt order (windows are emitted as
                their tiles become available)."""
                wins = [None] * (QW // P)

                def emit_win(j):
                    qb = qc * QW + j * P
                    r0 = max(0, qb - 512)
                    r1 = min(N, qb + 640)
                    rw = r1 - r0
                    wt = wpool.tile([P, 1152], bf16, name=f"win{j}", tag="win")
                    wsrc = AP(
                        tensor=wer_d[h][qc * (QW // P) + j],
                        offset=640 + r0 - qb,
                        ap=[[WER - 1, P], [1, rw]],
                    )
                    nc.gpsimd.dma_start(wt[:, :rw], wsrc)
                    wins[j] = (wt, r0)

                own_pending = [t for t in bg_tasks if t[0] == h]
                done_qt = set(range(NT)) - {t[1] for t in own_pending}

                def win_ready(j):
                    return (qc * (QW // P) + j) in done_qt

                def drain_until(j):
                    while not win_ready(j):
                        assert bg_tasks, f"window {j} has no pending wER task"
                        th, tq = bg_tasks.pop(0)
                        wer_task(th, tq, during_flash=True)
                        if th == h:
                            done_qt.add(tq)

                # emit windows available now; others as their tiles complete
                for j in range(QW // P):
                    if win_ready(j):
                        emit_win(j)

                oth = [
                    os_.tile([65, 512], f32, name=f"outT{half}", tag="outT")
                    for half in range(2)
                ]
                prev_at = None
                for kt in range(NT):
                    kb = kt * P
                    cls = []
                    for j in range(QW // P):
                        dlt = qc * QW + j * P - kb
                        cls.append(
                            "p" if dlt >= 640 else ("f" if dlt <= -640 else "w")
                        )
                    # windows needed this kt: emit any missing (draining bg)
                    for j in range(QW // P):
                        if cls[j] == "w" and wins[j] is None:
                            drain_until(j)
                            emit_win(j)
                    zh = []
                    for half in range(QW // 512):
                        j0 = half * 4
                        zt = zs.tile([P, 512], f32, name="zt", tag="zt")
                        runs = []
                        for j in range(j0, j0 + 4):
                            if runs and runs[-1][2] == cls[j]:
                                runs[-1][1] += P
                            else:
                                runs.append([(j - j0) * P, P, cls[j]])
                        first = True
                        for s, wd, c in runs:
                            kvar = {"p": ktp, "f": ktf, "w": kt2}[c]
                            nc.tensor.matmul(
                                zt[:, s : s + wd],
                                kvar[hs_of(h), kb : kb + P],
                                qt2[hs_of(h), qc * QW + j0 * P + s :
                                    qc * QW + j0 * P + s + wd],
                                start=first,
                                stop=False,
                                skip_group_check=True,
                            )
                            first = False
                        for j in range(j0, j0 + 4):
                            if cls[j] != "w":
                                continue
                            # transpose via normal matmul: wt_blk.T @ I —
                            # bf16 operands, accumulates into f32 PSUM
                            # (is_transpose would force a bf16 output)
                            wt, r0 = wins[j]
                            nc.tensor.matmul(
                                zt[:, (j - j0) * P : (j - j0 + 1) * P],
                                wt[:, kb - r0 : kb - r0 + P],
                                ident_sb[:],
                                start=False,
                                stop=False,
                                skip_group_check=True,
                            )
                        zh.append(zt)
                    at = apool.tile([P, QW], bf16, name="attnT", tag="attnT")
                    for half in range(2):
                        nc.scalar.activation(
                            at[:, half * 512 : (half + 1) * 512],
                            zh[half][:],
                            mybir.ActivationFunctionType.Exp,
                            scale=SCALE,
                        )
                    # lag AV by one kt so PE never stalls on the current exp
                    if prev_at is not None:
                        pat, pkt = prev_at
                        for half in range(2):
                            nc.tensor.matmul(
                                oth[half][:],
                                v2[:, h, pkt, :],
                                pat[:, half * 512 : (half + 1) * 512],
                                start=(pkt == 0),
                                stop=False,
                            )
                    prev_at = (at, kt)
                    # interleave background wER work between flash steps
                    if bg_tasks:
                        th, tq = bg_tasks.pop(0)
                        wer_task(th, tq, during_flash=True)
                        if th == h:
                            done_qt.add(tq)
                pat, pkt = prev_at
                for half in range(2):
                    nc.tensor.matmul(
                        oth[half][:],
                        v2[:, h, pkt, :],
                        pat[:, half * 512 : (half + 1) * 512],
                        start=False,
                        stop=True,
                    )
                return oth

            def hs_of(h):
                return slice(h * 64, h * 64 + 64)

            # wER h0 qt0..4 up front (flash(0,0) kt0 needs windows j0..4)
            for qt in range(5):
                wer_task(0, qt, during_flash=False)
            bg = [(0, qt) for qt in range(5, NT)]
            oth = emit_flash(0, 0, bg)
            emit_division(0, 0, oth, "v")
            bg = [(1, qt) for qt in range(NT)]
            oth = emit_flash(0, 1, bg)
            emit_division(0, 1, oth, "s")
            oth = emit_flash(1, 0, bg)
            emit_division(1, 0, oth, "v")
            emit_outproj(range(0, 8))  # rows 0..1023 ready; overlaps last flash
            oth = emit_flash(1, 1, bg)
            emit_division(1, 1, oth, "s")
            emit_outproj(range(8, 16))

    nc.compile()
    return nc


def _host_prep(x, Wq, Wkv, Wo, rel_emb):
    """Build the 8 per-core input maps."""
    import ml_dtypes

    bf = ml_dtypes.bfloat16
    ident = np.eye(P, dtype=np.float32).astype(bf)
    ones1 = np.ones((1, 64), dtype=bf)
    relX = rel_emb[np.clip(1152 - np.arange(WER), 0, 1024)].T
    relx2 = np.ascontiguousarray(np.concatenate([relX, relX], axis=0).astype(bf))
    edge = np.stack([rel_emb[1024], rel_emb[0]], axis=1)
    edge2 = np.ascontiguousarray(np.concatenate([edge, edge], axis=0).astype(np.float32))
    Wkv_r = Wkv.reshape(DIM, 2, HEADS, D)
    in_maps = []
    for core in range(8):
        b = core // 4
        h0 = 2 * (core % 4)
        in_maps.append(
            {
                "xT": np.ascontiguousarray(x[b].T.astype(bf)),
                "wq2": np.ascontiguousarray(Wq[:, h0 * D : (h0 + 2) * D].astype(bf)),
                "wk2": np.ascontiguousarray(
                    Wkv_r[:, 0, h0 : h0 + 2].reshape(DIM, 2 * D).astype(bf)
                ),
                "wv2": np.ascontiguousarray(
                    Wkv_r[:, 1, h0 : h0 + 2].reshape(DIM, 2 * D).astype(bf)
                ),
                "wo2": np.ascontiguousarray(
                    Wo[h0 * D : (h0 + 2) * D, :].astype(bf)
                ),
                "relx2": relx2,
                "edge2": edge2,
                "ident": ident,
                "ones1": ones1,
            }
        )
    return in_maps


def kernel(x, Wq, Wkv, Wo, bo, rel_emb, _want_trace=False):
    from concourse.bass_utils import run_bass_kernel_spmd

    x = np.asarray(x)
    if "nc" not in _cached:
        _cached["nc"] = _build_program()
    nc = _cached["nc"]
    in_maps = _host_prep(x, np.asarray(Wq), np.asarray(Wkv), np.asarray(Wo),
                         np.asarray(rel_emb))
    res = run_bass_kernel_spmd(
        nc, in_maps, core_ids=list(range(8)), trace=_want_trace
    )
    _cached["last_result"] = res
    y = np.zeros((2, N, DIM), np.float32)
    for core in range(8):
        y[core // 4] += res.results[core]["y"]
    y += np.asarray(bo).astype(np.float32)[None, None, :]
    return y
